# revision 25
# baseline (speedup 1.0000x reference)
"""Trainium2 Bass kernel: batched attention (B=8, S=4096, D=64), fp32.

out[b] = softmax(q[b] @ k[b].T / sqrt(D), axis=keys) @ v[b] * mask[b, :, None]

Sharding: ALL 8 batches run on ONE NeuronCore in a single NEFF. Measured on
this deployment, per-core NEFF launches serialize (per-call wall grows
~linearly with core count), so 8 single-batch dispatches cost 8x the fixed
per-launch + prologue overhead. One consolidated launch pays it once and
pipelines batches back-to-back on the device.

Per-batch algorithm (matmul operands in fp16; PSUM accumulation fp32):
  1. Stage q/k/v via DMA (batch b+1 staged while batch b computes), convert
     to fp16, transpose Q, K to d-major [64, S] via PE transpose (fp16:
     1 cycle/row), duplicate into partitions 64-127.
  2. scoresT[k, q] = K @ Q^T per (k-tile, q-chunk) slab into PSUM, with
     even/odd k-tiles issued to the two 64-row halves of the PE array --
     concurrent one-shot matmuls, ~132 ns/MM (HW-measured). NOTE: half-row
     matmuls must be one-shot; half-row PSUM accumulation interleaved across
     row groups is broken on HW.
  3. ScalarE reads PSUM slabs directly: PT = exp(0.125 * scoresT) -> SBUF
     fp16, in 3-slab chunks (amortizes the ~222-cycle ACT instruction
     overhead). No max subtraction: scaled scores are ~N(0,1), exp is safe.
     (A Schraudolph bit-trick exp on the DVE was tried to offload ACT --
     HW-measured SLOWER than all-ACT despite sim predicting a win (DVE
     tensor_scalar from PSUM underperforms; N_DVE knob kept for reference),
     and it cost accuracy: 1.6e-2 vs 7.7e-4.)
  4. PV: full-row (K=128) accumulating matmuls, stationary = [V_ktile | ones]
     (65 cols) so row 64 of the accumulated output is the softmax denominator
     for free. Full-row chains tolerate half-row one-shot interlopers
     (HW-verified).
  5. Epilogue: PE-transpose outT (+denominator row +mask row) back to natural
     [q, d] layout, fuse *mask/denom into the PSUM->SBUF copy, DMA out.
"""

import sys

if "/opt/trn_rl_repo" not in sys.path:
    sys.path.insert(0, "/opt/trn_rl_repo")

import math
from contextlib import ExitStack

import numpy as np

import concourse.bass as bass
import concourse.mybir as mybir
import concourse.tile as tile
from concourse import bacc
from concourse.masks import make_identity

F32 = mybir.dt.float32
FP16 = mybir.dt.float16
I16 = mybir.dt.int16

B = 8          # batch (all on one core)
S = 4096       # sequence length
D = 64         # head dim
P = 128        # partitions
NKT = S // P   # 32 k-tiles of 128 keys
QCHUNK = 512   # query chunk (one PSUM bank of fp32 per matmul)
NQC = S // QCHUNK          # 8 query chunks
GROUP = 16                 # k-tiles per PT group (PV consumes per group)
NGRP = NKT // GROUP        # 2 groups per q-chunk
SCALE = 1.0 / 8.0          # 1/sqrt(D)

# exp engine split per 16-slab group (("a", n) -> ACT LUT exp chunk of n
# slabs; ("d", n) -> DVE Schraudolph chunk). 3-slab chunks amortize the
# ~222-cycle ACT instruction overhead while fitting PSUM:
# 2 bufs x 3 banks + 1 PV + 1 shared(transpose/epilogue) = 8 banks.
import os
N_DVE = int(os.environ.get("KQ_N_DVE", "0"))
INTERLEAVE = os.environ.get("KQ_INTERLEAVE", "1") == "1"
EXP_SCHEDULE = {
    0: [("a", 3)] * 5 + [("a", 1)],
    3: [("a", 3)] * 4 + [("a", 1)] + [("d", 3)],
    4: [("a", 3)] * 4 + [("d", 3), ("d", 1)],
    5: [("a", 3)] * 3 + [("a", 2)] + [("d", 3), ("d", 2)],
    6: [("a", 3)] * 3 + [("a", 1)] + [("d", 3), ("d", 3)],
    7: [("a", 3)] * 3 + [("d", 3), ("d", 3), ("d", 1)],
    8: [("a", 3), ("a", 3), ("a", 2)] + [("d", 3), ("d", 3), ("d", 2)],
}
SCHRAUDOLPH_C = 60.0
A_DVE = float(np.float32(1024.0 * math.log2(math.e) / 8.0))
B_DVE = float(np.float32(15.0 * 1024.0 + 0.5 - SCHRAUDOLPH_C))


def build_attention(ctx: ExitStack, tc: tile.TileContext,
                    q_ap, k_ap, v_ap, mask_ap, out_ap, reps=1):
    nc = tc.nc
    sched = EXP_SCHEDULE[N_DVE]
    assert sum(n for _, n in sched) == GROUP

    const_pool = ctx.enter_context(tc.tile_pool(name="const", bufs=1))
    io_pool = ctx.enter_context(tc.tile_pool(name="io", bufs=1))

    ident = const_pool.tile([P, P], F32, tag="ident", name="ident")
    make_identity(nc, ident)
    ident16 = const_pool.tile([P, P], FP16, tag="ident16", name="ident16")
    make_identity(nc, ident16)

    # ---- persistent SBUF tensors (double-buffered across batches so batch
    # b+1's transposes/fills can run during batch b's main loop) ------------
    per_pool = ctx.enter_context(tc.tile_pool(name="per", bufs=2))

    def prepare_persistent():
        return {
            "qt": [per_pool.tile([P, S // 2], FP16, tag=f"qt{h}",
                                 name=f"qt{h}") for h in range(2)],
            "kt": [per_pool.tile([P, S // 2], FP16, tag=f"kt{h}",
                                 name=f"kt{h}") for h in range(2)],
            "vp": per_pool.tile([P, NKT, D + 1], FP16, tag="vp", name="vp"),
            "maskT": per_pool.tile([P, S // P], F32, tag="maskT",
                                   name="maskT"),
        }

    # ---- staging pools (double-buffered across batches) --------------------
    stage_pool = ctx.enter_context(tc.tile_pool(name="stage", bufs=2))
    ones = io_pool.tile([P, NKT], F32, tag="ones", name="ones")
    nc.gpsimd.memset(ones, 1.0)

    # ---- PSUM pools --------------------------------------------------------
    sc_pool = ctx.enter_context(tc.tile_pool(name="sc", bufs=2, space="PSUM"))
    pv_pool = ctx.enter_context(tc.tile_pool(name="pv", bufs=1, space="PSUM"))
    ps_pool = ctx.enter_context(tc.tile_pool(name="ps", bufs=1, space="PSUM"))
    # one shared PSUM bank: prologue transposes (fp16 view) + epilogue tiles
    ps_shared = ps_pool.tile([P, QCHUNK], F32, tag="ps", name="ps_shared")

    pt_pool = ctx.enter_context(tc.tile_pool(name="pt", bufs=2))
    outt_pool = ctx.enter_context(tc.tile_pool(name="outt", bufs=2))
    osb_pool = ctx.enter_context(tc.tile_pool(name="osb", bufs=2))
    scal_pool = ctx.enter_context(tc.tile_pool(name="scal", bufs=4))

    def stage(b):
        """Issue input DMAs for batch b into the rotating staging buffers."""
        qn = stage_pool.tile([P, NKT, D], F32, tag="qn", name="qn")
        kn = stage_pool.tile([P, NKT, D], F32, tag="kn", name="kn")
        vn = stage_pool.tile([P, NKT, D], F32, tag="vn", name="vn")
        o = b * S
        nc.sync.dma_start(
            qn[:], q_ap[o:o + S, :].rearrange("(t p) d -> p t d", p=P))
        nc.sync.dma_start(
            kn[:], k_ap[o:o + S, :].rearrange("(t p) d -> p t d", p=P))
        nc.sync.dma_start(
            vn[:], v_ap[o:o + S, :].rearrange("(t p) d -> p t d", p=P))
        return qn, kn, vn

    def convert(b, staged, cur):
        """fp16 conversions + vp/maskT fills for batch b (DVE + DMA only --
        keeps ScalarE free for exp, its steady-state bottleneck role)."""
        qn, kn, vn = staged
        qn16 = stage_pool.tile([P, NKT, D], FP16, tag="qn16", name="qn16")
        kn16 = stage_pool.tile([P, NKT, D], FP16, tag="kn16", name="kn16")
        nc.vector.tensor_copy(qn16[:], qn)
        nc.vector.tensor_copy(kn16[:], kn)
        vp = cur["vp"]
        nc.vector.tensor_copy(vp[:, :, D], ones)
        nc.vector.tensor_copy(vp[:, :, 0:D], vn)
        # mask transposed to [q%128, q//128] layout for the epilogue multiply
        nc.sync.dma_start(
            cur["maskT"],
            mask_ap[b:b + 1, :].rearrange("a (t p) -> p (a t)", p=P))
        return qn16, kn16

    N_TBLOCKS = 16

    def transpose_blocks(conv, cur, b0, n):
        """Transpose col-blocks b0..b0+n of batch-next's q/k through the
        shared PSUM bank (ping-pong halves). Every input tile is transposed
        TWICE -- into PSUM partitions 0-63 and 64-127 -- so one drain fills
        both SBUF row-halves (row-tiled matmuls need the data on both
        halves; beats the SBUF->SBUF duplicate DMA)."""
        qn16, kn16 = conv
        tps = ps_shared.bitcast(FP16)  # [128, 1024] fp16 view
        for hb in range(b0, b0 + n):
            half, src_i, c = hb // 8, (hb // 4) % 2, hb % 4
            src = qn16 if src_i == 0 else kn16
            dst = cur["qt" if src_i == 0 else "kt"]
            pp = tps[:, (hb % 2) * 4 * P:(hb % 2 + 1) * 4 * P]
            for j in range(4):
                t = half * (NKT // 2) + c * 4 + j
                nc.tensor.transpose(pp[0:D, j * P:(j + 1) * P],
                                    src[:, t, :], ident16)
                nc.tensor.transpose(pp[D:P, j * P:(j + 1) * P],
                                    src[:, t, :], ident16)
            nc.vector.tensor_copy(dst[half][:, c * 4 * P:(c + 1) * 4 * P], pp)

    pt_tiles = {}      # group index -> ptt tile
    pv_tiles = {}      # qc -> pv psum tile

    def emit_group(s, b, cur):
        """QK^T+exp for group s, with group s-1's PV matmuls interleaved
        between the exp chunks so ACT/DVE get a steady feed while PE works
        (all-QK-then-all-PV starves ACT during the PV phase)."""
        qt, kt, vp = cur["qt"], cur["kt"], cur["vp"]
        qc, g = divmod(s, NGRP)
        q0 = qc * QCHUNK
        qt_half = qt[(2 * q0) // S]
        qcol = q0 % (S // 2)
        ptt = pt_pool.tile([P, GROUP * QCHUNK], FP16, tag="ptt", name="ptt")
        pt_tiles[s] = ptt

        # previous group's PV state
        prev = s - 1
        pv_ps = prev_ptt = None
        if prev >= 0:
            pqc, pg = divmod(prev, NGRP)
            prev_ptt = pt_tiles.pop(prev)
            if pg == 0:
                # full-bank tile: rows 0-64 accumulate PV; after the drain
                # the same bank hosts the epilogue transposes
                pv_tiles[pqc] = pv_pool.tile([P, QCHUNK], F32, tag="pv",
                                             name="pv")
            pv_ps = pv_tiles[pqc]

        def pv_mm(jj):
            pqc, pg = divmod(prev, NGRP)
            k_tile = pg * GROUP + jj
            nc.tensor.matmul(
                pv_ps[0:D + 1, :],
                lhsT=vp[:, k_tile, :],
                rhs=prev_ptt[:, jj * QCHUNK:(jj + 1) * QCHUNK],
                start=(k_tile == 0), stop=(k_tile == NKT - 1),
                skip_group_check=True,
            )

        n_chunks = len(sched)
        pv_done = 0
        j = 0
        for ci, (eng, clen) in enumerate(sched):
            scs = sc_pool.tile([P, 3 * QCHUNK], F32, tag="sc", name="sc")
            for jj in range(j, j + clen):
                k_tile = g * GROUP + jj
                h = k_tile % 2  # row-tiling: alternate array halves
                kt_half = kt[(k_tile * P * 2) // S]
                kcol = (k_tile * P) % (S // 2)
                nc.tensor.matmul(
                    scs[:, (jj - j) * QCHUNK:(jj - j + 1) * QCHUNK],
                    lhsT=kt_half[h * D:(h + 1) * D, kcol:kcol + P],
                    rhs=qt_half[h * D:(h + 1) * D, qcol:qcol + QCHUNK],
                    start=True, stop=True,
                )
            dst = ptt[:, j * QCHUNK:(j + clen) * QCHUNK]
            if eng == "a":
                nc.scalar.activation(
                    dst, scs[:, 0:clen * QCHUNK],
                    mybir.ActivationFunctionType.Exp,
                    scale=SCALE,
                )
            else:
                nc.vector.tensor_scalar(
                    dst.bitcast(I16), scs[:, 0:clen * QCHUNK],
                    A_DVE, B_DVE,
                    mybir.AluOpType.mult, mybir.AluOpType.add,
                )
            j += clen
            # interleave a share of the previous group's PV matmuls
            if prev >= 0 and INTERLEAVE:
                want = (ci + 1) * GROUP // n_chunks
                for jj in range(pv_done, want):
                    pv_mm(jj)
                pv_done = want
        if prev >= 0 and not INTERLEAVE:
            for jj in range(GROUP):
                pv_mm(jj)
            pv_done = GROUP
        if prev >= 0:
            pqc, pg = divmod(prev, NGRP)
            if pg == NGRP - 1:
                emit_drain_epilogue(pqc, b, cur)

    def emit_last_pv(b, cur):
        """PV + epilogue for the final group of a batch (no successor)."""
        vp = cur["vp"]
        prev = n_groups - 1
        pqc, pg = divmod(prev, NGRP)
        ptt = pt_tiles.pop(prev)
        pv_ps = pv_tiles[pqc]
        for jj in range(GROUP):
            k_tile = pg * GROUP + jj
            nc.tensor.matmul(
                pv_ps[0:D + 1, :],
                lhsT=vp[:, k_tile, :],
                rhs=ptt[:, jj * QCHUNK:(jj + 1) * QCHUNK],
                start=(k_tile == 0), stop=(k_tile == NKT - 1),
                skip_group_check=True,
            )
        emit_drain_epilogue(pqc, b, cur)

    osb_cur = {}

    def emit_drain_epilogue(qc, b, cur):
        maskT = cur["maskT"]
        pv_ps = pv_tiles.pop(qc)
        outt = outt_pool.tile([D + 1, QCHUNK], F32, tag="outt", name="outt")
        # drain PV psum into outT staging (rows 0..63 out, row 64 denom)
        nc.vector.tensor_copy(outt[:], pv_ps[0:D + 1, :])
        # back to natural [q, d] layout; transposes reuse the (drained) PV
        # bank; output staged across 2 q-chunks to halve out-DMA count
        if qc % 2 == 0:
            osb_cur[0] = osb_pool.tile([P, 2 * (QCHUNK // P), D], F32,
                                       tag="osb", name="osb")
        osb = osb_cur[0]
        nt = QCHUNK // P  # 4 [128, 65] transposes per chunk
        tps4 = pv_ps[:, 0:nt * (D + 1)].rearrange(
            "p (j c) -> p j c", c=D + 1)
        for jj in range(nt):
            nc.tensor.transpose(tps4[:, jj, :], outt[:, jj * P:(jj + 1) * P],
                                ident[0:D + 1, 0:D + 1])
        # batched 1/denom * mask for all 4 sub-tiles (fewer DVE instructions)
        rs = scal_pool.tile([P, 2 * nt], F32, tag="rs", name="rs")
        nc.vector.reciprocal(rs[:, 0:nt], tps4[:, :, D])
        nc.vector.tensor_mul(rs[:, nt:2 * nt], rs[:, 0:nt],
                             maskT[:, qc * nt:(qc + 1) * nt])
        for jj in range(nt):
            nc.vector.tensor_scalar(
                osb[:, (qc % 2) * nt + jj, :], tps4[:, jj, 0:D],
                rs[:, nt + jj:nt + jj + 1], None,
                mybir.AluOpType.mult,
            )
        if qc % 2 == 1:
            nt = 2 * (QCHUNK // P)
            nc.sync.dma_start(
                out_ap[b * S:(b + 1) * S, :].rearrange(
                    "(t p) d -> p t d", p=P)[:, (qc - 1) * (QCHUNK // P):
                                             (qc + 1) * (QCHUNK // P), :], osb)

    # ---- batch loop --------------------------------------------------------
    # Batch b+1's staging DMA, fp16 converts, and PE transposes are spread
    # across batch b's groups (DVE/DMA slack), so batch boundaries cost only
    # the last-PV tail. Batch 0's prologue runs serially up front.
    loop_cm = None
    staged = stage(0)
    if reps > 1:
        loop_cm = tc.For_i(0, reps, 1, hint_engines=(
            mybir.EngineType.PE, mybir.EngineType.Activation,
            mybir.EngineType.DVE))
        loop_cm.__enter__()

    n_groups = NQC * NGRP
    cur = prepare_persistent()
    conv = convert(0, staged, cur)
    transpose_blocks(conv, cur, 0, N_TBLOCKS)
    nxt = conv_next = None
    for b in range(B):
        for s in range(n_groups):
            emit_group(s, b, cur)
            if b + 1 < B:
                if s == 2:
                    staged = stage(b + 1)
                    nxt = prepare_persistent()
                    qn16_n = stage_pool.tile([P, NKT, D], FP16, tag="qn16",
                                             name="qn16")
                    kn16_n = stage_pool.tile([P, NKT, D], FP16, tag="kn16",
                                             name="kn16")
                    conv_next = (qn16_n, kn16_n)
                    nc.vector.tensor_copy(qn16_n[:], staged[0])
                elif s == 3:
                    nc.vector.tensor_copy(conv_next[1][:], staged[1])
                elif s == 8:
                    vp_n = nxt["vp"]
                    nc.vector.tensor_copy(vp_n[:, :, D], ones)
                    nc.vector.tensor_copy(vp_n[:, :, 0:D], staged[2])
                    nc.sync.dma_start(
                        nxt["maskT"],
                        mask_ap[b + 1:b + 2, :].rearrange(
                            "a (t p) -> p (a t)", p=P))
                if 4 <= s < 12:
                    transpose_blocks(conv_next, nxt, (s - 4) * 2, 2)
        emit_last_pv(b, cur)
        if nxt is not None:
            cur = nxt

    if loop_cm is not None:
        loop_cm.__exit__(None, None, None)


def build_program(reps=1):
    nc = bacc.Bacc("TRN2", target_bir_lowering=False, debug=False,
                   num_devices=1)
    q = nc.declare_dram_parameter("q", [B * S, D], F32, isOutput=False).ap()
    k = nc.declare_dram_parameter("k", [B * S, D], F32, isOutput=False).ap()
    v = nc.declare_dram_parameter("v", [B * S, D], F32, isOutput=False).ap()
    mask = nc.declare_dram_parameter("mask", [B, S], F32, isOutput=False).ap()
    out = nc.declare_dram_parameter("out", [B * S, D], F32, isOutput=True).ap()

    with tile.TileContext(nc) as tc, ExitStack() as ctx:
        build_attention(ctx, tc, q, k, v, mask, out, reps=reps)
    nc.compile()
    return nc


_NC_CACHE = None


def _get_nc():
    global _NC_CACHE
    if _NC_CACHE is None:
        _NC_CACHE = build_program()
    return _NC_CACHE


def make_in_maps(q, k, v, mask):
    return [
        {
            "q": np.ascontiguousarray(q.reshape(B * S, D), dtype=np.float32),
            "k": np.ascontiguousarray(k.reshape(B * S, D), dtype=np.float32),
            "v": np.ascontiguousarray(v.reshape(B * S, D), dtype=np.float32),
            "mask": np.ascontiguousarray(mask, dtype=np.float32),
        }
    ]


def kernel(q, k, v, mask, _trace=False, _trace_kwargs=None):
    from concourse.bass_utils import run_bass_kernel_spmd

    nc = _get_nc()
    res = run_bass_kernel_spmd(
        nc, make_in_maps(q, k, v, mask), [0],
        trace=_trace, **(_trace_kwargs or {}),
    )
    out = res.results[0]["out"].reshape(B, S, D)
    if _trace:
        return out, res
    return out


if __name__ == "__main__":
    rng = np.random.default_rng(0)
    q = rng.standard_normal((B, S, D), dtype=np.float32)
    k = rng.standard_normal((B, S, D), dtype=np.float32)
    v = rng.standard_normal((B, S, D), dtype=np.float32)
    mask = np.ones((B, S), dtype=np.float32)
    out = kernel(q, k, v, mask)
    print("out", out.shape, out.dtype, float(np.abs(out).max()))
